# revision 39
# baseline (speedup 1.0000x reference)
"""Transformer encoder layer (B=4, S=2048, D=1024, H=16, FFN=4096) on 8 TRN2 cores.

Sharding: core c owns batch b=c//2, sequence half h=c%2 (1024 query tokens).

v3: query-half-major attention. Phase B processes qh=0 (512 queries, all
head-pairs) then qh=1, so PV accumulators need 2 PSUM banks instead of 4,
freeing 2 banks for a fill pool. The second half of the projections (V d1,
K for hp4-7) is emitted as always-ready matmul chains interleaved into
B-qh0's idle slots — the PE stream stays dense, which keeps the HAM clock
gate at full rate (the v2 kernel lost 2x clock through all of phase B to
HAM throttling from its ~56% PE duty cycle).

All matmuls bf16; all staging SBUF-resident; FFN2 accumulates over all 32
hidden tiles in PSUM.
"""
from contextlib import ExitStack

import numpy as np

import concourse.bass as bass
import concourse.tile as tile
from concourse import bacc, mybir
from concourse.bass_utils import run_bass_kernel_spmd
from concourse.masks import make_identity

F32 = mybir.dt.float32
BF16 = mybir.dt.bfloat16
MMDT = BF16
F8 = mybir.dt.float8e4
DR = mybir.MatmulPerfMode.DoubleRow

B, S, D, H, DH, HID = 4, 2048, 1024, 16, 64, 4096
SQ = S // 2           # query tokens per core
N_CORES = 8
LN_EPS = 1e-5
SCALE = 1.0 / np.sqrt(DH)
WS = 16.0             # fp8 weight pre-scale (host side)
# Wq,Wk scaled by WS each -> scores scaled WS^2; fold into exp scale
ESCALE = SCALE / (WS * WS)

KO = D // 128         # 8   contraction subtiles over D
KT = S // 128         # 16  key-token tiles
QT = SQ // 128        # 8   query-token tiles
HP = H // 2           # 8   head pairs
HT = HID // 128       # 32  hidden tiles
HB = 4                # hidden blocks (of 8 ht = 1024 hid each)

_BUILD_CACHE = {}


def _build(flags, debug=None):
    nc = bacc.Bacc("TRN2", target_bir_lowering=False, debug=False)

    XT = nc.dram_tensor("XT", [D, S], F8, kind="ExternalInput").ap()
    XQT = nc.dram_tensor("XQT", [D, SQ], F8, kind="ExternalInput").ap()
    XQ = nc.dram_tensor("XQ", [SQ, D], MMDT, kind="ExternalInput").ap()
    WQ = nc.dram_tensor("WQ", [D, D], F8, kind="ExternalInput").ap()
    WK = nc.dram_tensor("WK", [D, D], F8, kind="ExternalInput").ap()
    WV = nc.dram_tensor("WV", [D, D], F8, kind="ExternalInput").ap()
    WO = nc.dram_tensor("WO", [D, D], F8, kind="ExternalInput").ap()
    W1 = nc.dram_tensor("W1", [D, HID], MMDT, kind="ExternalInput").ap()
    W2 = nc.dram_tensor("W2", [HID, D], MMDT, kind="ExternalInput").ap()
    OUT = nc.dram_tensor("OUT", [SQ, D], F32, kind="ExternalOutput").ap()
    if debug == "ctx":
        CTXD = nc.dram_tensor("CTXD", [128, HP, SQ], F32, kind="ExternalOutput").ap()
    if debug == "y":
        YD = nc.dram_tensor("YD", [QT, 128, D], F32, kind="ExternalOutput").ap()

    ext = {}
    if "bqkv" in flags:
        for nm in ("BQ", "BK", "BV"):
            ext[nm] = nc.dram_tensor(nm, [D], F32, kind="ExternalInput").ap()
    if "bo" in flags:
        ext["BO"] = nc.dram_tensor("BO", [D], F32, kind="ExternalInput").ap()
    if "b1" in flags:
        ext["B1"] = nc.dram_tensor("B1", [HID], F32, kind="ExternalInput").ap()
    if "b2" in flags:
        ext["B2"] = nc.dram_tensor("B2", [D], F32, kind="ExternalInput").ap()
    if "g1b1" in flags:
        ext["G1"] = nc.dram_tensor("G1", [D], F32, kind="ExternalInput").ap()
        ext["BT1"] = nc.dram_tensor("BT1", [D], F32, kind="ExternalInput").ap()
    if "g2b2" in flags:
        ext["G2"] = nc.dram_tensor("G2", [D], F32, kind="ExternalInput").ap()
        ext["BT2"] = nc.dram_tensor("BT2", [D], F32, kind="ExternalInput").ap()

    def bcast_free(vec_ap, parts):
        return bass.AP(tensor=vec_ap.tensor, offset=vec_ap.offset,
                       ap=[[0, parts]] + list(vec_ap.ap))

    XTr = XT.rearrange("(ko p) t -> p ko t", p=128)
    XQTr = XQT.rearrange("(ko p) t -> p ko t", p=128)
    XQr = XQ.rearrange("(qt p) d -> p qt d", p=128)
    WKr = WK.rearrange("(ko p) d -> p ko d", p=128)
    WQr = WQ.rearrange("(ko p) d -> p ko d", p=128)
    WVr = WV.rearrange("(ko p) d -> p ko d", p=128)
    WOr = WO.rearrange("(ko p) d -> p ko d", p=128)
    W1r = W1.rearrange("(ko p) h -> p ko h", p=128)
    W2r = W2.rearrange("(ho p) d -> p ho d", p=128)
    OUTr = OUT.rearrange("(qt p) d -> qt p d", p=128)

    with tile.TileContext(nc) as tc, ExitStack() as ctx:
        persist = ctx.enter_context(tc.tile_pool(name="persist", bufs=1))

        eps_sb = persist.tile([128, 1], F32)
        nc.vector.memset(eps_sb[:], LN_EPS)
        ident = persist.tile([128, 128], F32)
        make_identity(nc, ident[:])
        ident_mm = persist.tile([128, 128], MMDT)
        nc.scalar.copy(ident_mm[:], ident[:])
        ones64_f = persist.tile([128, 64], F32)
        nc.vector.memset(ones64_f[:], 1.0)
        ones64 = persist.tile([128, 64], MMDT)
        nc.scalar.copy(ones64[:], ones64_f[:])

        if "bqkv" in flags:
            bq_sb = persist.tile([128, KO], F32)
            bk_sb = persist.tile([128, KO], F32)
            nc.sync.dma_start(bq_sb[:], ext["BQ"].rearrange("(o p) -> p o", p=128))
            nc.sync.dma_start(bk_sb[:], ext["BK"].rearrange("(o p) -> p o", p=128))
        if "b1" in flags:
            b1_sb = persist.tile([128, HT], F32)
            nc.sync.dma_start(b1_sb[:], ext["B1"].rearrange("(o p) -> p o", p=128))
        if "bo" in flags:
            bo_sb = persist.tile([128, D], F32)
            nc.sync.dma_start(bo_sb[:], bcast_free(ext["BO"], 128))
        if "b2" in flags:
            b2_sb = persist.tile([128, D], F32)
            nc.sync.dma_start(b2_sb[:], bcast_free(ext["B2"], 128))
        if "g1b1" in flags:
            g1_sb = persist.tile([128, D], F32)
            bt1_sb = persist.tile([128, D], F32)
            nc.sync.dma_start(g1_sb[:], bcast_free(ext["G1"], 128))
            nc.sync.dma_start(bt1_sb[:], bcast_free(ext["BT1"], 128))
        if "g2b2" in flags:
            g2_sb = persist.tile([128, D], F32)
            bt2_sb = persist.tile([128, D], F32)
            nc.sync.dma_start(g2_sb[:], bcast_free(ext["G2"], 128))
            nc.sync.dma_start(bt2_sb[:], bcast_free(ext["BT2"], 128))

        # ---------- persistent SBUF staging (LIFO lifetimes) ----------
        pCW = ctx.enter_context(tc.tile_pool(name="pC_w", bufs=1))
        wo_sb = pCW.tile([128, KO, D], F8)
        xq_sb = pCW.tile([128, QT, D], MMDT)
        pCTX = ctx.enter_context(tc.tile_pool(name="pCTX", bufs=1))
        ctxT = pCTX.tile([128, HP, SQ], F8)

        stk_kqv = ctx.enter_context(ExitStack())
        pKQV = stk_kqv.enter_context(tc.tile_pool(name="pKQV", bufs=1))
        ktsb = pKQV.tile([128, HP, S], MMDT)
        qtsb = pKQV.tile([128, HP, SQ], MMDT)
        vsb = pKQV.tile([128, KT, H, DH + 1], F8)

        stk_xt = ctx.enter_context(ExitStack())
        pXT = stk_xt.enter_context(tc.tile_pool(name="pXT", bufs=1))
        xt = pXT.tile([128, KO, S], F8)

        # Y staging (written by B-h1 fills + phase C, read in C/D); the
        # pool opens mid-B (after xt frees) — see below. pR1 (r1f/mvf
        # staging for qt0-3) closes after phase C.
        stk_y = ctx.enter_context(ExitStack())
        stk_r = ExitStack()
        ysb = yt = r1f = mvf = None

        # V ones column (sum-of-exp trick); value WS so the WS on V
        # cancels exactly in the softmax normalization
        onesc_f = persist.tile([128, KT * H], F32)
        nc.vector.memset(onesc_f[:], WS)
        nc.scalar.copy(vsb[:, :, :, DH:DH + 1],
                       onesc_f[:].rearrange("p (a b c) -> p a b c", a=KT, b=H))

        def rstd_ln(pool, var_ap, tagpfx, name):
            # 1/sqrt(var+eps): ACT Sqrt (bias folds the +eps) + exact DVE
            # reciprocal at FD=1. All rstds run in phases C/D, so the ACT
            # sqrt table set loads once and never swaps with exp's.
            sd = pool.tile([128, 1], F32, tag=tagpfx + "sd", name=name + "_sd")
            nc.scalar.activation(sd[:], var_ap,
                                 mybir.ActivationFunctionType.Sqrt,
                                 bias=eps_sb[:], scale=1.0)
            rstd = pool.tile([128, 1], F32, tag=tagpfx + "rs", name=name + "_rs")
            nc.vector.reciprocal(rstd[:], sd[:])
            return rstd

        def kcopy(dst, src, hp):
            if "bqkv" in flags:
                nc.scalar.activation(
                    dst, src, mybir.ActivationFunctionType.Identity,
                    bias=bk_sb[:, hp:hp + 1])
            else:
                nc.vector.tensor_copy(dst, src)

        def qcopy(dst, src, hp):
            if "bqkv" in flags:
                nc.scalar.activation(
                    dst, src, mybir.ActivationFunctionType.Identity,
                    bias=bq_sb[:, hp:hp + 1])
            else:
                nc.vector.tensor_copy(dst, src)

        # ---------- Phase A upfront: V(d0), K+Q(hp0-3), Q(hp4-7) ----------
        with (
            tc.tile_pool(name="pXQT", bufs=1) as pXQTp,
            tc.tile_pool(name="pA_w", bufs=2) as paw,
            tc.tile_pool(name="pA2_w", bufs=2) as pa2w,
            tc.tile_pool(name="psA2", bufs=6, space="PSUM") as psa2,
        ):
            xqt = pXQTp.tile([128, KO, SQ], F8)

            wv_h0 = paw.tile([128, KO, 512], F8, tag="wv")
            nc.sync.dma_start(wv_h0[:], WVr[:, :, 0:512])
            for c in range(8):
                nc.sync.dma_start(xt[:, :, c * 256:(c + 1) * 256],
                                  XTr[:, :, c * 256:(c + 1) * 256])
            for c in range(4):
                nc.sync.dma_start(xqt[:, :, c * 256:(c + 1) * 256],
                                  XQTr[:, :, c * 256:(c + 1) * 256])
            if "bqkv" in flags:
                bvb0 = paw.tile([128, 512], F32, tag="bv")
                nc.sync.dma_start(bvb0[:], bcast_free(ext["BV"][0:512], 128))

            KP = KO // 2      # fp8 DoubleRow: contract 2 ko-subtiles per MM
            for tt in range(KT):
                pvp = psa2.tile([128, 512], F32, tag="a", name=f"pv0_{tt}")
                for k in range(KP):
                    nc.tensor.matmul(
                        pvp[:], xt[:, 2 * k:2 * k + 2, tt * 128:(tt + 1) * 128],
                        wv_h0[:, 2 * k:2 * k + 2], start=(k == 0),
                        stop=(k == KP - 1), perf_mode=DR)
                vdst = vsb[:, tt, 0:8, 0:DH]
                pvp3 = pvp[:].rearrange("p (a b) -> p a b", a=8)
                if "bqkv" in flags:
                    nc.vector.tensor_add(
                        vdst, pvp3, bvb0[:].rearrange("p (a b) -> p a b", a=8))
                else:
                    nc.vector.tensor_copy(vdst, pvp3)

            for hp in range(4):
                wk_hp = pa2w.tile([128, KO, 128], F8, tag="wk")
                nc.sync.dma_start(wk_hp[:], WKr[:, :, hp * 128:(hp + 1) * 128])
                wq_hp = pa2w.tile([128, KO, 128], F8, tag="wq")
                nc.sync.dma_start(wq_hp[:], WQr[:, :, hp * 128:(hp + 1) * 128])
                for ns in range(S // 512):
                    pk = psa2.tile([128, 512], F32, tag="a", name=f"pk_{hp}_{ns}")
                    for k in range(KP):
                        nc.tensor.matmul(
                            pk[:], wk_hp[:, 2 * k:2 * k + 2],
                            xt[:, 2 * k:2 * k + 2, ns * 512:(ns + 1) * 512],
                            start=(k == 0), stop=(k == KP - 1), perf_mode=DR)
                    kcopy(ktsb[:, hp, ns * 512:(ns + 1) * 512], pk[:], hp)
                for ns in range(SQ // 512):
                    pq = psa2.tile([128, 512], F32, tag="a", name=f"pq_{hp}_{ns}")
                    for k in range(KP):
                        nc.tensor.matmul(
                            pq[:], wq_hp[:, 2 * k:2 * k + 2],
                            xqt[:, 2 * k:2 * k + 2, ns * 512:(ns + 1) * 512],
                            start=(k == 0), stop=(k == KP - 1), perf_mode=DR)
                    qcopy(qtsb[:, hp, ns * 512:(ns + 1) * 512], pq[:], hp)
            for hp in range(4, 8):
                wq_h = pa2w.tile([128, KO, 128], F8, tag="wq",
                                 name=f"wq_{hp}")
                nc.sync.dma_start(wq_h[:], WQr[:, :, hp * 128:(hp + 1) * 128])
                for ns in range(SQ // 512):
                    pq = psa2.tile([128, 512], F32, tag="a", name=f"pq_{hp}_{ns}")
                    for k in range(KP):
                        nc.tensor.matmul(
                            pq[:], wq_h[:, 2 * k:2 * k + 2],
                            xqt[:, 2 * k:2 * k + 2, ns * 512:(ns + 1) * 512],
                            start=(k == 0), stop=(k == KP - 1), perf_mode=DR)
                    qcopy(qtsb[:, hp, ns * 512:(ns + 1) * 512], pq[:], hp)

        # ---------- Phase B: attention, query-half-major, with fills ----------
        if debug != "ctx":
            nc.sync.dma_start(wo_sb[:], WOr)
            nc.sync.dma_start(xq_sb[:], XQr)

        with (
            tc.tile_pool(name="psF", bufs=2, space="PSUM") as psf,
            tc.tile_pool(name="psS", bufs=2, space="PSUM") as pss,
            tc.tile_pool(name="psPV", bufs=2, space="PSUM") as pspv,
        ):
            def attn_half(qh, pbp, pbst, pbn, pump):
                q0 = qh * 512
                pending_norm = []

                def emit_norm(hp, pv_ps):
                    for h in range(2):
                        stage = pbst.tile([DH + 1, 512], MMDT, tag="stage",
                                          name=f"stg_{qh}_{hp}_{h}")
                        nc.vector.tensor_copy(stage[:], pv_ps[h][:])
                        bc = psf.tile([64, 512], F32, tag="fill",
                                      name=f"bc_{qh}_{hp}_{h}")
                        nc.tensor.matmul(bc[:], ones64[64:65, :],
                                         stage[64:65, :], start=True, stop=True)
                        rb = pbn.tile([64, 512], F32, tag="rb",
                                      name=f"rb_{qh}_{hp}_{h}")
                        nc.vector.reciprocal_approx_fast(rb[:], bc[:])
                        if h == 0:
                            nc.vector.tensor_mul(
                                ctxT[0:64, hp, q0:q0 + 512],
                                stage[0:DH], rb[:])
                        else:
                            tmp1 = pbn.tile([64, 512], MMDT, tag="tmp1",
                                            name=f"tmp1_{qh}_{hp}")
                            nc.vector.tensor_mul(tmp1[:], stage[0:DH], rb[:])
                            nc.gpsimd.dma_start(
                                ctxT[64:128, hp, q0:q0 + 512], tmp1[:])

                for hp in range(HP):
                    pv_ps = [pspv.tile([DH + 1, 512], F32, tag="pv",
                                       name=f"pv_{qh}_{hp}_{h}")
                             for h in range(2)]
                    pair_tiles = {}

                    def pv_pair(ktp, hp=hp, pv_ps=pv_ps,
                                pair_tiles=pair_tiles):
                        # fp8 DoubleRow: contract both kt tiles of the pair
                        # (256 keys) in one matmul per head
                        pp = pair_tiles.pop(ktp)
                        for h in range(2):
                            nc.tensor.matmul(
                                pv_ps[h][:],
                                vsb[:, 2 * ktp:2 * ktp + 2, 2 * hp + h, :],
                                pp[:, :, h, :],
                                start=(ktp == 0), stop=(ktp == KT // 2 - 1),
                                perf_mode=DR, skip_group_check=True)

                    for kt in range(KT):
                        s_t = pss.tile([128, 2, 512], F32, tag="s",
                                       name=f"s_{qh}_{hp}_{kt}")
                        for h in range(2):
                            nc.tensor.matmul(
                                s_t[:, h],
                                ktsb[h * 64:(h + 1) * 64, hp,
                                     kt * 128:(kt + 1) * 128],
                                qtsb[h * 64:(h + 1) * 64, hp, q0:q0 + 512],
                                start=True, stop=True)
                        # fills go right after S in the in-order PE queue, so
                        # they run in the slot where PE would wait for exp
                        pump(hp, kt)
                        # previous head-pair's normalization is deferred past
                        # this hp's first S/exp so its bc matmuls don't stall
                        # the exp stream at the boundary
                        if kt == 0 and pending_norm:
                            pending_norm.pop()()
                        if kt % 2 == 0:
                            pp = pbp.tile([128, 2, 2, 512], F8, tag="p",
                                          name=f"p_{qh}_{hp}_{kt // 2}")
                            pair_tiles[kt // 2] = pp
                        else:
                            pp = pair_tiles[kt // 2]
                        nc.scalar.activation(
                            pp[:, kt % 2].rearrange("p a b -> p (a b)"),
                            s_t[:].rearrange("p a b -> p (a b)"),
                            mybir.ActivationFunctionType.Exp,
                            bias=0.0, scale=float(ESCALE))
                        if kt % 2 == 1 and kt // 2 > 0:
                            pv_pair(kt // 2 - 1)
                    pv_pair(KT // 2 - 1)
                    pending_norm.append(
                        lambda hp=hp, pv_ps=pv_ps: emit_norm(hp, pv_ps))
                pending_norm.pop()()

            # --- B half 0: fills = V(d1) + K(hp4-7) chains ---
            with (
                tc.tile_pool(name="pAW2", bufs=1) as paw2,
                tc.tile_pool(name="pB_p0", bufs=3) as pbp0,
                tc.tile_pool(name="pB_st0", bufs=4) as pbst0,
                tc.tile_pool(name="pB_n0", bufs=4) as pbn0,
            ):
                fill_q = []
                KP = KO // 2
                wv_d1 = paw2.tile([128, KO, 512], F8, name="wv_d1")
                nc.sync.dma_start(wv_d1[:], WVr[:, :, 512:1024])
                if "bqkv" in flags:
                    bvb1 = paw2.tile([128, 512], F32, name="bvb1")
                    nc.sync.dma_start(bvb1[:], bcast_free(ext["BV"][512:1024], 128))
                wk_fill = {}
                for hp in range(4, 8):
                    wk_t = paw2.tile([128, KO, 128], F8, name=f"wk_f{hp}")
                    nc.sync.dma_start(wk_t[:], WKr[:, :, hp * 128:(hp + 1) * 128])
                    wk_fill[hp] = wk_t

                for tt in range(KT):
                    def mk_v(tt=tt):
                        pvp = psf.tile([128, 512], F32, tag="fill",
                                       name=f"fv_{tt}")
                        for k in range(KP):
                            nc.tensor.matmul(
                                pvp[:],
                                xt[:, 2 * k:2 * k + 2, tt * 128:(tt + 1) * 128],
                                wv_d1[:, 2 * k:2 * k + 2], start=(k == 0),
                                stop=(k == KP - 1), perf_mode=DR)
                        vdst = vsb[:, tt, 8:16, 0:DH]
                        pvp3 = pvp[:].rearrange("p (a b) -> p a b", a=8)
                        if "bqkv" in flags:
                            nc.vector.tensor_add(
                                vdst, pvp3,
                                bvb1[:].rearrange("p (a b) -> p a b", a=8))
                        else:
                            nc.vector.tensor_copy(vdst, pvp3)
                    fill_q.append(mk_v)
                for hp in range(4, 8):
                    for ns in range(S // 512):
                        def mk_k(hp=hp, ns=ns):
                            pk = psf.tile([128, 512], F32, tag="fill",
                                          name=f"fk_{hp}_{ns}")
                            for k in range(KP):
                                nc.tensor.matmul(
                                    pk[:], wk_fill[hp][:, 2 * k:2 * k + 2],
                                    xt[:, 2 * k:2 * k + 2,
                                       ns * 512:(ns + 1) * 512],
                                    start=(k == 0), stop=(k == KP - 1),
                                    perf_mode=DR)
                            kcopy(ktsb[:, hp, ns * 512:(ns + 1) * 512],
                                  pk[:], hp)
                        fill_q.append(mk_k)

                def pump0(hp, kt):
                    if hp < 4 and kt % 2 == 0 and fill_q:
                        fill_q.pop(0)()

                attn_half(0, pbp0, pbst0, pbn0, pump0)
                while fill_q:
                    fill_q.pop(0)()

            stk_xt.close()  # free xt
            pY = stk_y.enter_context(
                tc.tile_pool(name="pY", bufs=1, side="right"))
            ysb = pY.tile([128, QT, D], MMDT)
            yt = pY.tile([128, KO, SQ], MMDT)
            pR = stk_r.enter_context(
                tc.tile_pool(name="pR1", bufs=1, side="right"))
            r1f = pR.tile([128, 4, D], F32)   # qt0-3 residual (B-h1 fills)
            mvf = pR.tile([128, 4, 2], F32)   # qt0-3 LN1 mean/var

            # --- B half 1: fills = out-proj + residual + LN stats for
            # qt0-3 (PE/DVE only; no ACT so the exp stream never swaps
            # table sets; LN scale + transposes are deferred to phase C) ---
            with (
                tc.tile_pool(name="pB_p1", bufs=4) as pbp1,
                tc.tile_pool(name="pB_st1", bufs=6) as pbst1,
                tc.tile_pool(name="pB_n1", bufs=4) as pbn1,
                tc.tile_pool(name="pC1", bufs=2) as pc1,
            ):
                cfill_q = []
                if debug != "ctx":
                    for qt in range(4):
                        def mk_po(qt=qt, dh=0):
                            po = psf.tile([128, 512], F32, tag="fill",
                                          name=f"fpo_{qt}_{dh}")
                            for hq in range(HP // 2):
                                nc.tensor.matmul(
                                    po[:],
                                    ctxT[:, 2 * hq:2 * hq + 2,
                                         qt * 128:(qt + 1) * 128],
                                    wo_sb[:, 2 * hq:2 * hq + 2,
                                          dh * 512:(dh + 1) * 512],
                                    start=(hq == 0), stop=(hq == HP // 2 - 1),
                                    perf_mode=DR)
                            nc.vector.tensor_add(
                                r1f[:, qt, dh * 512:(dh + 1) * 512], po[:],
                                xq_sb[:, qt, dh * 512:(dh + 1) * 512])
                            if dh == 1:
                                r1 = r1f[:, qt]
                                if "bo" in flags:
                                    nc.vector.tensor_add(r1, r1, bo_sb[:])
                                stats = pc1.tile([128, 2, 6], F32, tag="stf",
                                                 name=f"stf_{qt}")
                                r1v = r1.rearrange("p (s d) -> p s d", s=2)
                                for sgi in range(2):
                                    nc.vector.bn_stats(stats[:, sgi],
                                                       r1v[:, sgi])
                                nc.vector.bn_aggr(mvf[:, qt], stats[:])
                        cfill_q.append(lambda qt=qt: mk_po(qt, 0))
                        cfill_q.append(lambda qt=qt: mk_po(qt, 1))

                def pump1(hp, kt):
                    # spread the 8 C-fill items across the whole half so
                    # every head-pair's idle gets some PE work
                    if (hp * KT + kt) % 12 == 0 and cfill_q:
                        cfill_q.pop(0)()

                attn_half(1, pbp1, pbst1, pbn1, pump1)
                while cfill_q:
                    cfill_q.pop(0)()

        stk_kqv.close()  # free ktsb/qtsb/vsb

        if debug == "ctx":
            with tc.tile_pool(name="dbg", bufs=2) as dbg:
                for hp in range(HP):
                    t = dbg.tile([128, SQ], F32)
                    nc.vector.tensor_copy(t[:], ctxT[:, hp, :])
                    nc.sync.dma_start(CTXD[:, hp, :], t[:])
            out_stub = persist.tile([128, 1], F32)
            nc.vector.memset(out_stub[:], 0.0)
            nc.sync.dma_start(OUT[0:1, 0:128].rearrange("a b -> b a"), out_stub[:])

        phase_cd = debug != "ctx"
        phase_d = debug is None

        # ---------- Phase C: LN1 + y^T (DMA transpose) + out-proj qt4-7;
        # FFN1 qb=0 overlaps the qt4-7 normalization ----------
        def ln_scale_transpose(qt, r1, mean_ap, var_ap, pool):
            rstd = rstd_ln(pool, var_ap, "c", f"rsc_{qt}")
            ytile = ysb[:, qt, :]
            nc.vector.tensor_scalar(
                ytile, r1, scalar1=mean_ap, scalar2=rstd[:],
                op0=mybir.AluOpType.subtract, op1=mybir.AluOpType.mult)
            if "g1b1" in flags:
                nc.vector.tensor_mul(ytile, ytile, g1_sb[:])
                nc.vector.tensor_add(ytile, ytile, bt1_sb[:])
            for dt in range(KO):
                nc.sync.dma_start_transpose(
                    yt[:, dt, qt * 128:(qt + 1) * 128],
                    ysb[:, qt, dt * 128:(dt + 1) * 128])

        stk_f = ctx.enter_context(ExitStack())
        pF = stk_f.enter_context(tc.tile_pool(name="pF", bufs=1))
        fsb = pF.tile([128, HT, SQ], MMDT, name="fsb") if phase_d else None
        pwD = ctx.enter_context(tc.tile_pool(name="pD_w", bufs=2))

        with (
            tc.tile_pool(name="pC_s", bufs=4) as pcs,
            tc.tile_pool(name="psC", bufs=3, space="PSUM") as psc,
            tc.tile_pool(name="psD", bufs=3, space="PSUM",
                         side="right") as psd,
        ):
          if phase_cd:
            # part 2a: LN scale + transposes for qt0-3 (stats staged in B-h1)
            for qt in range(4):
                ln_scale_transpose(qt, r1f[:, qt], mvf[:, qt, 0:1],
                                   mvf[:, qt, 1:2], pcs)
            # part 1: out-proj + residual + stats for qt4-7 (PE-heavy,
            # runs while the DVE does part 2a)
            r1c, mvc = {}, {}
            for qt in range(4, QT):
                r1t = pcs.tile([128, D], F32, tag="r1", name=f"r1_{qt}")
                for dh in range(2):
                    po = psc.tile([128, 512], F32)
                    for hq in range(HP // 2):
                        nc.tensor.matmul(
                            po[:],
                            ctxT[:, 2 * hq:2 * hq + 2,
                                 qt * 128:(qt + 1) * 128],
                            wo_sb[:, 2 * hq:2 * hq + 2,
                                  dh * 512:(dh + 1) * 512],
                            start=(hq == 0), stop=(hq == HP // 2 - 1),
                            perf_mode=DR)
                    nc.vector.tensor_add(
                        r1t[:, dh * 512:(dh + 1) * 512], po[:],
                        xq_sb[:, qt, dh * 512:(dh + 1) * 512])
                if "bo" in flags:
                    nc.vector.tensor_add(r1t[:], r1t[:], bo_sb[:])
                stats = pcs.tile([128, 2, 6], F32, tag="st")
                r1v = r1t[:].rearrange("p (s d) -> p s d", s=2)
                for sgi in range(2):
                    nc.vector.bn_stats(stats[:, sgi], r1v[:, sgi])
                mvt = pcs.tile([128, 2], F32, tag="mv", name=f"mv_{qt}")
                nc.vector.bn_aggr(mvt[:], stats[:])
                r1c[qt], mvc[qt] = r1t, mvt

            # FFN1 qb=0 (query tiles 0-3): starts as soon as the qt0-3
            # transposes land, overlapping part 2b's normalization
            def ffn1_pass(qb):
                for hb in range(HB):
                    w1_hb = pwD.tile([128, KO, 1024], MMDT, tag="w",
                                     name=f"w1_{qb}_{hb}")
                    nc.sync.dma_start(
                        w1_hb[:], W1r[:, :, hb * 1024:(hb + 1) * 1024])
                    for hti in range(8):
                        pf = psd.tile([128, 512], F32, tag="pf",
                                      name=f"pf_{qb}_{hb}_{hti}")
                        for k in range(KO):
                            nc.tensor.matmul(
                                pf[:], w1_hb[:, k, hti * 128:(hti + 1) * 128],
                                yt[:, k, qb * 512:(qb + 1) * 512],
                                start=(k == 0), stop=(k == KO - 1))
                        fdst = fsb[:, hb * 8 + hti, qb * 512:(qb + 1) * 512]
                        if "b1" in flags:
                            nc.vector.tensor_scalar(
                                fdst, pf[:],
                                scalar1=b1_sb[:, hb * 8 + hti:hb * 8 + hti + 1],
                                scalar2=0.0,
                                op0=mybir.AluOpType.add,
                                op1=mybir.AluOpType.max)
                        else:
                            nc.vector.tensor_scalar(
                                fdst, pf[:], scalar1=0.0, scalar2=None,
                                op0=mybir.AluOpType.max,
                                op1=mybir.AluOpType.bypass)

            if phase_d:
                ffn1_pass(0)
            # part 2b: LN scale + transposes for qt4-7
            for qt in range(4, QT):
                ln_scale_transpose(qt, r1c[qt][:], mvc[qt][:, 0:1],
                                   mvc[qt][:, 1:2], pcs)
            if phase_d:
                ffn1_pass(1)

        stk_r.close()  # free r1f/mvf

        if debug == "y":
            with tc.tile_pool(name="dbg2", bufs=2) as dbg2:
                for qt in range(QT):
                    t = dbg2.tile([128, D], F32)
                    nc.vector.tensor_copy(t[:], ysb[:, qt, :])
                    nc.sync.dma_start(YD[qt], t[:])
            out_stub2 = persist.tile([128, 1], F32)
            nc.vector.memset(out_stub2[:], 0.0)
            nc.sync.dma_start(OUT[0:1, 0:128].rearrange("a b -> b a"), out_stub2[:])

        with (
            tc.tile_pool(name="pD_s", bufs=3) as pds,
            tc.tile_pool(name="psD2", bufs=8, space="PSUM") as psd2,
        ):
          if phase_d:
            for qtg, (g0, g1) in enumerate([(0, 4), (4, 8)]):
                p2 = [[psd2.tile([128, 512], F32, tag="p2",
                                 name=f"p2_{qtg}_{lqt}_{dhh}")
                       for dhh in range(2)] for lqt in range(g1 - g0)]
                def drain(lqt):
                    qt = g0 + lqt
                    r2 = pds.tile([128, D], F32, tag="r2", name=f"r2_{qt}")
                    for dhh in range(2):
                        nc.vector.tensor_add(
                            r2[:, dhh * 512:(dhh + 1) * 512], p2[lqt][dhh][:],
                            ysb[:, qt, dhh * 512:(dhh + 1) * 512])
                    if "b2" in flags:
                        nc.vector.tensor_add(r2[:], r2[:], b2_sb[:])
                    stats = pds.tile([128, 2, 6], F32, tag="st2")
                    r2v = r2[:].rearrange("p (s d) -> p s d", s=2)
                    for sgi in range(2):
                        nc.vector.bn_stats(stats[:, sgi], r2v[:, sgi])
                    mv = pds.tile([128, 2], F32, tag="mv2")
                    nc.vector.bn_aggr(mv[:], stats[:])
                    rstd = rstd_ln(pds, mv[:, 1:2], "d", f"rsd_{qt}")
                    o = pds.tile([128, D], F32, tag="o")
                    nc.vector.tensor_scalar(
                        o[:], r2[:], scalar1=mv[:, 0:1], scalar2=rstd[:],
                        op0=mybir.AluOpType.subtract, op1=mybir.AluOpType.mult)
                    if "g2b2" in flags:
                        nc.vector.tensor_mul(o[:], o[:], g2_sb[:])
                        nc.vector.tensor_add(o[:], o[:], bt2_sb[:])
                    nc.gpsimd.dma_start(OUTr[qt], o[:])

                for hb in range(HB):
                    w2_hb = pwD.tile([128, 8, D], MMDT, tag="w",
                                     name=f"w2_{qtg}_{hb}")
                    nc.sync.dma_start(w2_hb[:], W2r[:, hb * 8:(hb + 1) * 8, :])
                    last = hb == HB - 1
                    for lqt in range(g1 - g0):
                        qt = g0 + lqt
                        for dhh in range(2):
                            for hti in range(8):
                                nc.tensor.matmul(
                                    p2[lqt][dhh][:],
                                    fsb[:, hb * 8 + hti,
                                        qt * 128:(qt + 1) * 128],
                                    w2_hb[:, hti, dhh * 512:(dhh + 1) * 512],
                                    start=(hb == 0 and hti == 0),
                                    stop=(last and hti == 7),
                                    skip_group_check=True)
                        if last:
                            # drain each qt as soon as its accumulation stops
                            # so the LN/output overlaps the remaining MMs
                            drain(lqt)

    nc.compile()
    return nc


def _get_program(flags, debug=None):
    key = (flags, debug)
    if key not in _BUILD_CACHE:
        _BUILD_CACHE[key] = _build(flags, debug)
    return _BUILD_CACHE[key]


def _mm_np(a):
    import ml_dtypes
    return np.ascontiguousarray(a, dtype=ml_dtypes.bfloat16)


def _f8_np(a, scale=1.0):
    import ml_dtypes
    if scale != 1.0:
        a = np.asarray(a, np.float32) * scale
    return np.ascontiguousarray(a, dtype=ml_dtypes.float8_e4m3fn)


def _make_in_maps(X, shared):
    in_maps = []
    for c in range(N_CORES):
        b, half = c // 2, c % 2
        xq = np.ascontiguousarray(X[b, half * SQ:(half + 1) * SQ])
        m = dict(shared)
        # XQ (residual) is pre-scaled by WS to match po = ctx @ (WS*Wo);
        # LN1 is scale-invariant so Y is unchanged
        m.update({"XT": _f8_np(X[b].T),
                  "XQT": _f8_np(xq.T), "XQ": _mm_np(xq * WS)})
        in_maps.append(m)
    return in_maps


def kernel(X, Wq, bq, Wk, bk, Wv, bv, Wo, bo, g1, beta1, W1, b1, W2, b2, g2,
           beta2, _debug=None, _trace=False):
    f32 = lambda a: np.ascontiguousarray(np.asarray(a), dtype=np.float32)
    X = f32(X)
    Wq, Wk, Wv, Wo, W1, W2 = map(f32, (Wq, Wk, Wv, Wo, W1, W2))
    bq, bk, bv, bo, b1, b2 = map(f32, (bq, bk, bv, bo, b1, b2))
    g1, beta1, g2, beta2 = map(f32, (g1, beta1, g2, beta2))

    flags = set()
    if bq.any() or bk.any() or bv.any():
        flags.add("bqkv")
    if bo.any():
        flags.add("bo")
    if b1.any():
        flags.add("b1")
    if b2.any():
        flags.add("b2")
    if (g1 != 1).any() or beta1.any():
        flags.add("g1b1")
    if (g2 != 1).any() or beta2.any():
        flags.add("g2b2")
    flags = frozenset(flags)

    nc = _get_program(flags, _debug)

    shared = {"WQ": _f8_np(Wq, WS), "WK": _f8_np(Wk, WS),
              "WV": _f8_np(Wv, WS), "WO": _f8_np(Wo, WS),
              "W1": _mm_np(W1), "W2": _mm_np(W2)}
    if "bqkv" in flags:
        shared.update({"BQ": bq, "BK": bk, "BV": bv})
    if "bo" in flags:
        shared["BO"] = bo
    if "b1" in flags:
        shared["B1"] = b1
    if "b2" in flags:
        shared["B2"] = b2
    if "g1b1" in flags:
        shared.update({"G1": g1, "BT1": beta1})
    if "g2b2" in flags:
        shared.update({"G2": g2, "BT2": beta2})

    in_maps = _make_in_maps(X, shared)
    res = run_bass_kernel_spmd(nc, in_maps, core_ids=list(range(N_CORES)),
                               trace=_trace)

    if _debug is not None or _trace:
        return res

    out = np.empty((B, S, D), dtype=np.float32)
    for c in range(N_CORES):
        b, half = c // 2, c % 2
        out[b, half * SQ:(half + 1) * SQ] = res.results[c]["OUT"]
    return out



# revision 42
# speedup vs baseline: 1.0763x; 1.0763x over previous
"""Transformer encoder layer (B=4, S=2048, D=1024, H=16, FFN=4096) on 8 TRN2 cores.

Sharding: core c owns batch b=c//2, sequence half h=c%2 (1024 query tokens).

v3: query-half-major attention. Phase B processes qh=0 (512 queries, all
head-pairs) then qh=1, so PV accumulators need 2 PSUM banks instead of 4,
freeing 2 banks for a fill pool. The second half of the projections (V d1,
K for hp4-7) is emitted as always-ready matmul chains interleaved into
B-qh0's idle slots — the PE stream stays dense, which keeps the HAM clock
gate at full rate (the v2 kernel lost 2x clock through all of phase B to
HAM throttling from its ~56% PE duty cycle).

All matmuls bf16; all staging SBUF-resident; FFN2 accumulates over all 32
hidden tiles in PSUM.
"""
from contextlib import ExitStack

import numpy as np

import concourse.bass as bass
import concourse.tile as tile
from concourse import bacc, mybir
from concourse.bass_utils import run_bass_kernel_spmd
from concourse.masks import make_identity

F32 = mybir.dt.float32
BF16 = mybir.dt.bfloat16
MMDT = BF16
F8 = mybir.dt.float8e4
DR = mybir.MatmulPerfMode.DoubleRow

B, S, D, H, DH, HID = 4, 2048, 1024, 16, 64, 4096
SQ = S // 2           # query tokens per core
N_CORES = 8
LN_EPS = 1e-5
SCALE = 1.0 / np.sqrt(DH)
WS = 16.0             # fp8 weight pre-scale (host side)
# Wq,Wk scaled by WS each -> scores scaled WS^2; fold into exp scale
ESCALE = SCALE / (WS * WS)

KO = D // 128         # 8   contraction subtiles over D
KT = S // 128         # 16  key-token tiles
QT = SQ // 128        # 8   query-token tiles
HP = H // 2           # 8   head pairs
HT = HID // 128       # 32  hidden tiles
HB = 4                # hidden blocks (of 8 ht = 1024 hid each)

_BUILD_CACHE = {}


def _build(flags, debug=None):
    nc = bacc.Bacc("TRN2", target_bir_lowering=False, debug=False)

    XT = nc.dram_tensor("XT", [D, S], F8, kind="ExternalInput").ap()
    XQT = nc.dram_tensor("XQT", [D, SQ], F8, kind="ExternalInput").ap()
    XQ = nc.dram_tensor("XQ", [SQ, D], MMDT, kind="ExternalInput").ap()
    WQ = nc.dram_tensor("WQ", [D, D], F8, kind="ExternalInput").ap()
    WK = nc.dram_tensor("WK", [D, D], F8, kind="ExternalInput").ap()
    WV = nc.dram_tensor("WV", [D, D], F8, kind="ExternalInput").ap()
    WO = nc.dram_tensor("WO", [D, D], F8, kind="ExternalInput").ap()
    W1 = nc.dram_tensor("W1", [D, HID], MMDT, kind="ExternalInput").ap()
    W2 = nc.dram_tensor("W2", [HID, D], MMDT, kind="ExternalInput").ap()
    OUT = nc.dram_tensor("OUT", [SQ, D], F32, kind="ExternalOutput").ap()
    if debug == "ctx":
        CTXD = nc.dram_tensor("CTXD", [128, HP, SQ], F32, kind="ExternalOutput").ap()
    if debug == "y":
        YD = nc.dram_tensor("YD", [QT, 128, D], F32, kind="ExternalOutput").ap()

    ext = {}
    if "bqkv" in flags:
        for nm in ("BQ", "BK", "BV"):
            ext[nm] = nc.dram_tensor(nm, [D], F32, kind="ExternalInput").ap()
    if "bo" in flags:
        ext["BO"] = nc.dram_tensor("BO", [D], F32, kind="ExternalInput").ap()
    if "b1" in flags:
        ext["B1"] = nc.dram_tensor("B1", [HID], F32, kind="ExternalInput").ap()
    if "b2" in flags:
        ext["B2"] = nc.dram_tensor("B2", [D], F32, kind="ExternalInput").ap()
    if "g1b1" in flags:
        ext["G1"] = nc.dram_tensor("G1", [D], F32, kind="ExternalInput").ap()
        ext["BT1"] = nc.dram_tensor("BT1", [D], F32, kind="ExternalInput").ap()
    if "g2b2" in flags:
        ext["G2"] = nc.dram_tensor("G2", [D], F32, kind="ExternalInput").ap()
        ext["BT2"] = nc.dram_tensor("BT2", [D], F32, kind="ExternalInput").ap()

    def bcast_free(vec_ap, parts):
        return bass.AP(tensor=vec_ap.tensor, offset=vec_ap.offset,
                       ap=[[0, parts]] + list(vec_ap.ap))

    XTr = XT.rearrange("(ko p) t -> p ko t", p=128)
    XQTr = XQT.rearrange("(ko p) t -> p ko t", p=128)
    XQr = XQ.rearrange("(qt p) d -> p qt d", p=128)
    WKr = WK.rearrange("(ko p) d -> p ko d", p=128)
    WQr = WQ.rearrange("(ko p) d -> p ko d", p=128)
    WVr = WV.rearrange("(ko p) d -> p ko d", p=128)
    WOr = WO.rearrange("(ko p) d -> p ko d", p=128)
    W1r = W1.rearrange("(ko p) h -> p ko h", p=128)
    W2r = W2.rearrange("(ho p) d -> p ho d", p=128)
    OUTr = OUT.rearrange("(qt p) d -> qt p d", p=128)

    with tile.TileContext(nc) as tc, ExitStack() as ctx:
        persist = ctx.enter_context(tc.tile_pool(name="persist", bufs=1))

        eps_sb = persist.tile([128, 1], F32)
        nc.vector.memset(eps_sb[:], LN_EPS)
        ident = persist.tile([128, 128], F32)
        make_identity(nc, ident[:])
        ident_mm = persist.tile([128, 128], MMDT)
        nc.scalar.copy(ident_mm[:], ident[:])
        ones64_f = persist.tile([128, 64], F32)
        nc.vector.memset(ones64_f[:], 1.0)
        ones64 = persist.tile([128, 64], MMDT)
        nc.scalar.copy(ones64[:], ones64_f[:])

        if "bqkv" in flags:
            bq_sb = persist.tile([128, KO], F32)
            bk_sb = persist.tile([128, KO], F32)
            nc.sync.dma_start(bq_sb[:], ext["BQ"].rearrange("(o p) -> p o", p=128))
            nc.sync.dma_start(bk_sb[:], ext["BK"].rearrange("(o p) -> p o", p=128))
        if "b1" in flags:
            b1_sb = persist.tile([128, HT], F32)
            nc.sync.dma_start(b1_sb[:], ext["B1"].rearrange("(o p) -> p o", p=128))
        if "bo" in flags:
            bo_sb = persist.tile([128, D], F32)
            nc.sync.dma_start(bo_sb[:], bcast_free(ext["BO"], 128))
        if "b2" in flags:
            b2_sb = persist.tile([128, D], F32)
            nc.sync.dma_start(b2_sb[:], bcast_free(ext["B2"], 128))
        if "g1b1" in flags:
            g1_sb = persist.tile([128, D], F32)
            bt1_sb = persist.tile([128, D], F32)
            nc.sync.dma_start(g1_sb[:], bcast_free(ext["G1"], 128))
            nc.sync.dma_start(bt1_sb[:], bcast_free(ext["BT1"], 128))
        if "g2b2" in flags:
            g2_sb = persist.tile([128, D], F32)
            bt2_sb = persist.tile([128, D], F32)
            nc.sync.dma_start(g2_sb[:], bcast_free(ext["G2"], 128))
            nc.sync.dma_start(bt2_sb[:], bcast_free(ext["BT2"], 128))

        # ---------- persistent SBUF staging (LIFO lifetimes) ----------
        pCW = ctx.enter_context(tc.tile_pool(name="pC_w", bufs=1))
        wo_sb = pCW.tile([128, KO, D], F8)
        xq_sb = pCW.tile([128, QT, D], MMDT)
        pCTX = ctx.enter_context(tc.tile_pool(name="pCTX", bufs=1))
        ctxT = pCTX.tile([128, HP, SQ], F8)

        stk_kqv = ctx.enter_context(ExitStack())
        pKQV = stk_kqv.enter_context(tc.tile_pool(name="pKQV", bufs=1))
        ktsb = pKQV.tile([128, HP, S], MMDT)
        qtsb = pKQV.tile([128, HP, SQ], MMDT)
        vsb = pKQV.tile([128, KT, H, DH + 1], F8)

        stk_xt = ctx.enter_context(ExitStack())
        pXT = stk_xt.enter_context(tc.tile_pool(name="pXT", bufs=1))
        xt = pXT.tile([128, KO, S], F8)

        # Y staging (written by B-h1 fills + phase C, read in C/D); the
        # pool opens mid-B (after xt frees) — see below. pR1 (r1f/mvf
        # staging for qt0-3) closes after phase C.
        stk_y = ctx.enter_context(ExitStack())
        stk_r = ExitStack()
        ysb = yt = r1f = mvf = None

        # V ones column (sum-of-exp trick); value WS so the WS on V
        # cancels exactly in the softmax normalization
        onesc_f = persist.tile([128, KT * H], F32)
        nc.vector.memset(onesc_f[:], WS)
        nc.scalar.copy(vsb[:, :, :, DH:DH + 1],
                       onesc_f[:].rearrange("p (a b c) -> p a b c", a=KT, b=H))

        def rstd_ln(pool, var_ap, tagpfx, name):
            # 1/sqrt(var+eps): ACT Sqrt (bias folds the +eps) + exact DVE
            # reciprocal at FD=1. All rstds run in phases C/D, so the ACT
            # sqrt table set loads once and never swaps with exp's.
            sd = pool.tile([128, 1], F32, tag=tagpfx + "sd", name=name + "_sd")
            nc.scalar.activation(sd[:], var_ap,
                                 mybir.ActivationFunctionType.Sqrt,
                                 bias=eps_sb[:], scale=1.0)
            rstd = pool.tile([128, 1], F32, tag=tagpfx + "rs", name=name + "_rs")
            nc.vector.reciprocal(rstd[:], sd[:])
            return rstd

        def kcopy(dst, src, hp):
            if "bqkv" in flags:
                nc.scalar.activation(
                    dst, src, mybir.ActivationFunctionType.Identity,
                    bias=bk_sb[:, hp:hp + 1])
            else:
                nc.vector.tensor_copy(dst, src)

        def qcopy(dst, src, hp):
            if "bqkv" in flags:
                nc.scalar.activation(
                    dst, src, mybir.ActivationFunctionType.Identity,
                    bias=bq_sb[:, hp:hp + 1])
            else:
                nc.vector.tensor_copy(dst, src)

        # ---------- Phase A upfront: V(d0), K+Q(hp0-3), Q(hp4-7) ----------
        with (
            tc.tile_pool(name="pXQT", bufs=1) as pXQTp,
            tc.tile_pool(name="pA_w", bufs=2) as paw,
            tc.tile_pool(name="pA2_w", bufs=2) as pa2w,
            tc.tile_pool(name="psA2", bufs=6, space="PSUM") as psa2,
        ):
            xqt = pXQTp.tile([128, KO, SQ], F8)

            wv_h0 = paw.tile([128, KO, 512], F8, tag="wv")
            nc.sync.dma_start(wv_h0[:], WVr[:, :, 0:512])
            for c in range(8):
                nc.sync.dma_start(xt[:, :, c * 256:(c + 1) * 256],
                                  XTr[:, :, c * 256:(c + 1) * 256])
            for c in range(4):
                nc.sync.dma_start(xqt[:, :, c * 256:(c + 1) * 256],
                                  XQTr[:, :, c * 256:(c + 1) * 256])
            if "bqkv" in flags:
                bvb0 = paw.tile([128, 512], F32, tag="bv")
                nc.sync.dma_start(bvb0[:], bcast_free(ext["BV"][0:512], 128))

            KP = KO // 2      # fp8 DoubleRow: contract 2 ko-subtiles per MM
            for tt in range(KT):
                pvp = psa2.tile([128, 512], F32, tag="a", name=f"pv0_{tt}")
                for k in range(KP):
                    nc.tensor.matmul(
                        pvp[:], xt[:, 2 * k:2 * k + 2, tt * 128:(tt + 1) * 128],
                        wv_h0[:, 2 * k:2 * k + 2], start=(k == 0),
                        stop=(k == KP - 1), perf_mode=DR)
                vdst = vsb[:, tt, 0:8, 0:DH]
                pvp3 = pvp[:].rearrange("p (a b) -> p a b", a=8)
                if "bqkv" in flags:
                    nc.vector.tensor_add(
                        vdst, pvp3, bvb0[:].rearrange("p (a b) -> p a b", a=8))
                else:
                    nc.vector.tensor_copy(vdst, pvp3)

            for hp in range(4):
                wk_hp = pa2w.tile([128, KO, 128], F8, tag="wk")
                nc.sync.dma_start(wk_hp[:], WKr[:, :, hp * 128:(hp + 1) * 128])
                wq_hp = pa2w.tile([128, KO, 128], F8, tag="wq")
                nc.sync.dma_start(wq_hp[:], WQr[:, :, hp * 128:(hp + 1) * 128])
                for ns in range(S // 512):
                    pk = psa2.tile([128, 512], F32, tag="a", name=f"pk_{hp}_{ns}")
                    for k in range(KP):
                        nc.tensor.matmul(
                            pk[:], wk_hp[:, 2 * k:2 * k + 2],
                            xt[:, 2 * k:2 * k + 2, ns * 512:(ns + 1) * 512],
                            start=(k == 0), stop=(k == KP - 1), perf_mode=DR)
                    kcopy(ktsb[:, hp, ns * 512:(ns + 1) * 512], pk[:], hp)
                for ns in range(SQ // 512):
                    pq = psa2.tile([128, 512], F32, tag="a", name=f"pq_{hp}_{ns}")
                    for k in range(KP):
                        nc.tensor.matmul(
                            pq[:], wq_hp[:, 2 * k:2 * k + 2],
                            xqt[:, 2 * k:2 * k + 2, ns * 512:(ns + 1) * 512],
                            start=(k == 0), stop=(k == KP - 1), perf_mode=DR)
                    qcopy(qtsb[:, hp, ns * 512:(ns + 1) * 512], pq[:], hp)
            for hp in range(4, 8):
                wq_h = pa2w.tile([128, KO, 128], F8, tag="wq",
                                 name=f"wq_{hp}")
                nc.sync.dma_start(wq_h[:], WQr[:, :, hp * 128:(hp + 1) * 128])
                for ns in range(SQ // 512):
                    pq = psa2.tile([128, 512], F32, tag="a", name=f"pq_{hp}_{ns}")
                    for k in range(KP):
                        nc.tensor.matmul(
                            pq[:], wq_h[:, 2 * k:2 * k + 2],
                            xqt[:, 2 * k:2 * k + 2, ns * 512:(ns + 1) * 512],
                            start=(k == 0), stop=(k == KP - 1), perf_mode=DR)
                    qcopy(qtsb[:, hp, ns * 512:(ns + 1) * 512], pq[:], hp)

        # ---------- Phase B: attention, query-half-major, with fills ----------
        if debug != "ctx":
            nc.sync.dma_start(wo_sb[:], WOr)
            nc.sync.dma_start(xq_sb[:], XQr)

        with (
            tc.tile_pool(name="psF", bufs=2, space="PSUM") as psf,
            tc.tile_pool(name="psS", bufs=2, space="PSUM") as pss,
            tc.tile_pool(name="psPV", bufs=2, space="PSUM") as pspv,
        ):
            def attn_half(qh, pbp, pbst, pbn, pump):
                q0 = qh * 512
                pending_norm = []

                def emit_norm(hp, pv_ps):
                    for h in range(2):
                        stage = pbst.tile([DH + 1, 512], MMDT, tag="stage",
                                          name=f"stg_{qh}_{hp}_{h}")
                        nc.vector.tensor_copy(stage[:], pv_ps[h][:])
                        bc = psf.tile([64, 512], F32, tag="fill",
                                      name=f"bc_{qh}_{hp}_{h}")
                        nc.tensor.matmul(bc[:], ones64[64:65, :],
                                         stage[64:65, :], start=True, stop=True)
                        rb = pbn.tile([64, 512], F32, tag="rb",
                                      name=f"rb_{qh}_{hp}_{h}")
                        nc.vector.reciprocal_approx_fast(rb[:], bc[:])
                        if h == 0:
                            nc.vector.tensor_mul(
                                ctxT[0:64, hp, q0:q0 + 512],
                                stage[0:DH], rb[:])
                        else:
                            tmp1 = pbn.tile([64, 512], MMDT, tag="tmp1",
                                            name=f"tmp1_{qh}_{hp}")
                            nc.vector.tensor_mul(tmp1[:], stage[0:DH], rb[:])
                            nc.gpsimd.dma_start(
                                ctxT[64:128, hp, q0:q0 + 512], tmp1[:])

                for hp in range(HP):
                    pv_ps = [pspv.tile([DH + 1, 512], F32, tag="pv",
                                       name=f"pv_{qh}_{hp}_{h}")
                             for h in range(2)]
                    pair_tiles = {}

                    def pv_pair(ktp, hp=hp, pv_ps=pv_ps,
                                pair_tiles=pair_tiles):
                        # fp8 DoubleRow: contract both kt tiles of the pair
                        # (256 keys) in one matmul per head
                        pp = pair_tiles.pop(ktp)
                        for h in range(2):
                            nc.tensor.matmul(
                                pv_ps[h][:],
                                vsb[:, 2 * ktp:2 * ktp + 2, 2 * hp + h, :],
                                pp[:, :, h, :],
                                start=(ktp == 0), stop=(ktp == KT // 2 - 1),
                                perf_mode=DR, skip_group_check=True)

                    for kt in range(KT):
                        s_t = pss.tile([128, 2, 512], F32, tag="s",
                                       name=f"s_{qh}_{hp}_{kt}")
                        for h in range(2):
                            nc.tensor.matmul(
                                s_t[:, h],
                                ktsb[h * 64:(h + 1) * 64, hp,
                                     kt * 128:(kt + 1) * 128],
                                qtsb[h * 64:(h + 1) * 64, hp, q0:q0 + 512],
                                start=True, stop=True)
                        # fills go right after S in the in-order PE queue, so
                        # they run in the slot where PE would wait for exp
                        pump(hp, kt)
                        # previous head-pair's normalization is deferred past
                        # this hp's first S/exp so its bc matmuls don't stall
                        # the exp stream at the boundary
                        if kt == 0 and pending_norm:
                            pending_norm.pop()()
                        if kt % 2 == 0:
                            pp = pbp.tile([128, 2, 2, 512], F8, tag="p",
                                          name=f"p_{qh}_{hp}_{kt // 2}")
                            pair_tiles[kt // 2] = pp
                        else:
                            pp = pair_tiles[kt // 2]
                        nc.scalar.activation(
                            pp[:, kt % 2].rearrange("p a b -> p (a b)"),
                            s_t[:].rearrange("p a b -> p (a b)"),
                            mybir.ActivationFunctionType.Exp,
                            bias=0.0, scale=float(ESCALE))
                        if kt % 2 == 1 and kt // 2 > 0:
                            pv_pair(kt // 2 - 1)
                    pv_pair(KT // 2 - 1)
                    pending_norm.append(
                        lambda hp=hp, pv_ps=pv_ps: emit_norm(hp, pv_ps))
                pending_norm.pop()()

            # --- B half 0: fills = V(d1) + K(hp4-7) chains ---
            with (
                tc.tile_pool(name="pAW2", bufs=1) as paw2,
                tc.tile_pool(name="pB_p0", bufs=3) as pbp0,
                tc.tile_pool(name="pB_st0", bufs=4) as pbst0,
                tc.tile_pool(name="pB_n0", bufs=4) as pbn0,
            ):
                fill_q = []
                KP = KO // 2
                wv_d1 = paw2.tile([128, KO, 512], F8, name="wv_d1")
                nc.sync.dma_start(wv_d1[:], WVr[:, :, 512:1024])
                if "bqkv" in flags:
                    bvb1 = paw2.tile([128, 512], F32, name="bvb1")
                    nc.sync.dma_start(bvb1[:], bcast_free(ext["BV"][512:1024], 128))
                wk_fill = {}
                for hp in range(4, 8):
                    wk_t = paw2.tile([128, KO, 128], F8, name=f"wk_f{hp}")
                    nc.sync.dma_start(wk_t[:], WKr[:, :, hp * 128:(hp + 1) * 128])
                    wk_fill[hp] = wk_t

                for tt in range(KT):
                    def mk_v(tt=tt):
                        pvp = psf.tile([128, 512], F32, tag="fill",
                                       name=f"fv_{tt}")
                        for k in range(KP):
                            nc.tensor.matmul(
                                pvp[:],
                                xt[:, 2 * k:2 * k + 2, tt * 128:(tt + 1) * 128],
                                wv_d1[:, 2 * k:2 * k + 2], start=(k == 0),
                                stop=(k == KP - 1), perf_mode=DR)
                        vdst = vsb[:, tt, 8:16, 0:DH]
                        pvp3 = pvp[:].rearrange("p (a b) -> p a b", a=8)
                        if "bqkv" in flags:
                            nc.vector.tensor_add(
                                vdst, pvp3,
                                bvb1[:].rearrange("p (a b) -> p a b", a=8))
                        else:
                            nc.vector.tensor_copy(vdst, pvp3)
                    fill_q.append(mk_v)
                for hp in range(4, 8):
                    for ns in range(S // 512):
                        def mk_k(hp=hp, ns=ns):
                            pk = psf.tile([128, 512], F32, tag="fill",
                                          name=f"fk_{hp}_{ns}")
                            for k in range(KP):
                                nc.tensor.matmul(
                                    pk[:], wk_fill[hp][:, 2 * k:2 * k + 2],
                                    xt[:, 2 * k:2 * k + 2,
                                       ns * 512:(ns + 1) * 512],
                                    start=(k == 0), stop=(k == KP - 1),
                                    perf_mode=DR)
                            kcopy(ktsb[:, hp, ns * 512:(ns + 1) * 512],
                                  pk[:], hp)
                        fill_q.append(mk_k)

                def pump0(hp, kt):
                    if hp < 4 and kt % 2 == 0 and fill_q:
                        fill_q.pop(0)()

                attn_half(0, pbp0, pbst0, pbn0, pump0)
                while fill_q:
                    fill_q.pop(0)()

            stk_xt.close()  # free xt
            pY = stk_y.enter_context(
                tc.tile_pool(name="pY", bufs=1, side="right"))
            ysb = pY.tile([128, QT, D], MMDT)
            yt = pY.tile([128, KO, SQ], MMDT)
            pR = stk_r.enter_context(
                tc.tile_pool(name="pR1", bufs=1, side="right"))
            r1f = pR.tile([128, 4, D], F32)   # qt0-3 residual (B-h1 fills)
            mvf = pR.tile([128, 4, 2], F32)   # qt0-3 LN1 mean/var

            # --- B half 1: fills = out-proj + residual + LN stats for
            # qt0-3 (PE/DVE only; no ACT so the exp stream never swaps
            # table sets; LN scale + transposes are deferred to phase C) ---
            with (
                tc.tile_pool(name="pB_p1", bufs=4) as pbp1,
                tc.tile_pool(name="pB_st1", bufs=6) as pbst1,
                tc.tile_pool(name="pB_n1", bufs=4) as pbn1,
                tc.tile_pool(name="pC1", bufs=2) as pc1,
            ):
                cfill_q = []
                if debug != "ctx":
                    for qt in range(4):
                        def mk_po(qt=qt, dh=0):
                            po = psf.tile([128, 512], F32, tag="fill",
                                          name=f"fpo_{qt}_{dh}")
                            for hq in range(HP // 2):
                                nc.tensor.matmul(
                                    po[:],
                                    ctxT[:, 2 * hq:2 * hq + 2,
                                         qt * 128:(qt + 1) * 128],
                                    wo_sb[:, 2 * hq:2 * hq + 2,
                                          dh * 512:(dh + 1) * 512],
                                    start=(hq == 0), stop=(hq == HP // 2 - 1),
                                    perf_mode=DR)
                            nc.vector.tensor_add(
                                r1f[:, qt, dh * 512:(dh + 1) * 512], po[:],
                                xq_sb[:, qt, dh * 512:(dh + 1) * 512])
                            if dh == 1:
                                r1 = r1f[:, qt]
                                if "bo" in flags:
                                    nc.vector.tensor_add(r1, r1, bo_sb[:])
                                stats = pc1.tile([128, 2, 6], F32, tag="stf",
                                                 name=f"stf_{qt}")
                                r1v = r1.rearrange("p (s d) -> p s d", s=2)
                                for sgi in range(2):
                                    nc.vector.bn_stats(stats[:, sgi],
                                                       r1v[:, sgi])
                                nc.vector.bn_aggr(mvf[:, qt], stats[:])
                        cfill_q.append(lambda qt=qt: mk_po(qt, 0))
                        cfill_q.append(lambda qt=qt: mk_po(qt, 1))

                def pump1(hp, kt):
                    # spread the 8 C-fill items across the whole half so
                    # every head-pair's idle gets some PE work
                    if (hp * KT + kt) % 12 == 0 and cfill_q:
                        cfill_q.pop(0)()

                attn_half(1, pbp1, pbst1, pbn1, pump1)
                while cfill_q:
                    cfill_q.pop(0)()

        stk_kqv.close()  # free ktsb/qtsb/vsb

        if debug == "ctx":
            with tc.tile_pool(name="dbg", bufs=2) as dbg:
                for hp in range(HP):
                    t = dbg.tile([128, SQ], F32)
                    nc.vector.tensor_copy(t[:], ctxT[:, hp, :])
                    nc.sync.dma_start(CTXD[:, hp, :], t[:])
            out_stub = persist.tile([128, 1], F32)
            nc.vector.memset(out_stub[:], 0.0)
            nc.sync.dma_start(OUT[0:1, 0:128].rearrange("a b -> b a"), out_stub[:])

        phase_cd = debug != "ctx"
        phase_d = debug is None

        # ---------- Phase C: LN1 + y^T + out-proj qt4-7; FFN1 qb=0
        # overlaps the qt4-7 normalization ----------
        def ln_scale_transpose(qt, r1, mean_ap, rstd, pool, pst):
            ytile = ysb[:, qt, :]
            nc.vector.tensor_scalar(
                ytile, r1, scalar1=mean_ap, scalar2=rstd[:],
                op0=mybir.AluOpType.subtract, op1=mybir.AluOpType.mult)
            if "g1b1" in flags:
                nc.vector.tensor_mul(ytile, ytile, g1_sb[:])
                nc.vector.tensor_add(ytile, ytile, bt1_sb[:])
            for dt in range(KO):
                ptp = pst.tile([128, 128], MMDT, tag="tp")
                nc.tensor.transpose(
                    ptp[:], ysb[:, qt, dt * 128:(dt + 1) * 128], ident_mm[:])
                nc.vector.tensor_copy(
                    yt[:, dt, qt * 128:(qt + 1) * 128], ptp[:])

        stk_f = ctx.enter_context(ExitStack())
        pF = stk_f.enter_context(tc.tile_pool(name="pF", bufs=1))
        fsb = pF.tile([128, HT, SQ], MMDT, name="fsb") if phase_d else None
        pwD = ctx.enter_context(tc.tile_pool(name="pD_w", bufs=2))

        with (
            tc.tile_pool(name="pC_s", bufs=4) as pcs,
            tc.tile_pool(name="psC", bufs=2, space="PSUM") as psc,
            tc.tile_pool(name="psT", bufs=2, space="PSUM") as pst,
            tc.tile_pool(name="psD", bufs=3, space="PSUM",
                         side="right") as psd,
        ):
          if phase_cd:
            # rstds for qt0-3 first (tiny ACT+DVE, stats staged in B-h1)
            rstd03 = [rstd_ln(pcs, mvf[:, qt, 1:2], "c", f"rsc_{qt}")
                      for qt in range(4)]
            # part 1: out-proj + residual + stats for qt4-7 (PE-heavy,
            # runs while the DVE does the qt0-3 normalization)
            r1c, mvc = {}, {}
            for qt in range(4, QT):
                r1t = pcs.tile([128, D], F32, tag="r1", name=f"r1_{qt}")
                for dh in range(2):
                    po = psc.tile([128, 512], F32)
                    for hq in range(HP // 2):
                        nc.tensor.matmul(
                            po[:],
                            ctxT[:, 2 * hq:2 * hq + 2,
                                 qt * 128:(qt + 1) * 128],
                            wo_sb[:, 2 * hq:2 * hq + 2,
                                  dh * 512:(dh + 1) * 512],
                            start=(hq == 0), stop=(hq == HP // 2 - 1),
                            perf_mode=DR)
                    nc.vector.tensor_add(
                        r1t[:, dh * 512:(dh + 1) * 512], po[:],
                        xq_sb[:, qt, dh * 512:(dh + 1) * 512])
                if "bo" in flags:
                    nc.vector.tensor_add(r1t[:], r1t[:], bo_sb[:])
                stats = pcs.tile([128, 2, 6], F32, tag="st")
                r1v = r1t[:].rearrange("p (s d) -> p s d", s=2)
                for sgi in range(2):
                    nc.vector.bn_stats(stats[:, sgi], r1v[:, sgi])
                mvt = pcs.tile([128, 2], F32, tag="mv", name=f"mv_{qt}")
                nc.vector.bn_aggr(mvt[:], stats[:])
                r1c[qt], mvc[qt] = r1t, mvt

            # part 2a: LN scale + transposes for qt0-3
            for qt in range(4):
                ln_scale_transpose(qt, r1f[:, qt], mvf[:, qt, 0:1],
                                   rstd03[qt], pcs, pst)

            # FFN1 qb=0 (query tiles 0-3): starts as soon as the qt0-3
            # transposes land, overlapping part 2b's normalization
            def ffn1_pass(qb):
                for hb in range(HB):
                    w1_hb = pwD.tile([128, KO, 1024], MMDT, tag="w",
                                     name=f"w1_{qb}_{hb}")
                    nc.sync.dma_start(
                        w1_hb[:], W1r[:, :, hb * 1024:(hb + 1) * 1024])
                    for hti in range(8):
                        pf = psd.tile([128, 512], F32, tag="pf",
                                      name=f"pf_{qb}_{hb}_{hti}")
                        for k in range(KO):
                            nc.tensor.matmul(
                                pf[:], w1_hb[:, k, hti * 128:(hti + 1) * 128],
                                yt[:, k, qb * 512:(qb + 1) * 512],
                                start=(k == 0), stop=(k == KO - 1))
                        fdst = fsb[:, hb * 8 + hti, qb * 512:(qb + 1) * 512]
                        if "b1" in flags:
                            nc.vector.tensor_scalar(
                                fdst, pf[:],
                                scalar1=b1_sb[:, hb * 8 + hti:hb * 8 + hti + 1],
                                scalar2=0.0,
                                op0=mybir.AluOpType.add,
                                op1=mybir.AluOpType.max)
                        else:
                            nc.vector.tensor_scalar(
                                fdst, pf[:], scalar1=0.0, scalar2=None,
                                op0=mybir.AluOpType.max,
                                op1=mybir.AluOpType.bypass)

            if phase_d:
                ffn1_pass(0)
            # part 2b: LN scale + transposes for qt4-7
            for qt in range(4, QT):
                rstd = rstd_ln(pcs, mvc[qt][:, 1:2], "c", f"rsc_{qt}")
                ln_scale_transpose(qt, r1c[qt][:], mvc[qt][:, 0:1],
                                   rstd, pcs, pst)
            if phase_d:
                ffn1_pass(1)

        stk_r.close()  # free r1f/mvf

        if debug == "y":
            with tc.tile_pool(name="dbg2", bufs=2) as dbg2:
                for qt in range(QT):
                    t = dbg2.tile([128, D], F32)
                    nc.vector.tensor_copy(t[:], ysb[:, qt, :])
                    nc.sync.dma_start(YD[qt], t[:])
            out_stub2 = persist.tile([128, 1], F32)
            nc.vector.memset(out_stub2[:], 0.0)
            nc.sync.dma_start(OUT[0:1, 0:128].rearrange("a b -> b a"), out_stub2[:])

        with (
            tc.tile_pool(name="pD_s", bufs=3) as pds,
            tc.tile_pool(name="psD2", bufs=8, space="PSUM") as psd2,
        ):
          if phase_d:
            for qtg, (g0, g1) in enumerate([(0, 4), (4, 8)]):
                p2 = [[psd2.tile([128, 512], F32, tag="p2",
                                 name=f"p2_{qtg}_{lqt}_{dhh}")
                       for dhh in range(2)] for lqt in range(g1 - g0)]
                def drain(lqt):
                    qt = g0 + lqt
                    r2 = pds.tile([128, D], F32, tag="r2", name=f"r2_{qt}")
                    for dhh in range(2):
                        nc.vector.tensor_add(
                            r2[:, dhh * 512:(dhh + 1) * 512], p2[lqt][dhh][:],
                            ysb[:, qt, dhh * 512:(dhh + 1) * 512])
                    if "b2" in flags:
                        nc.vector.tensor_add(r2[:], r2[:], b2_sb[:])
                    stats = pds.tile([128, 2, 6], F32, tag="st2")
                    r2v = r2[:].rearrange("p (s d) -> p s d", s=2)
                    for sgi in range(2):
                        nc.vector.bn_stats(stats[:, sgi], r2v[:, sgi])
                    mv = pds.tile([128, 2], F32, tag="mv2")
                    nc.vector.bn_aggr(mv[:], stats[:])
                    rstd = rstd_ln(pds, mv[:, 1:2], "d", f"rsd_{qt}")
                    o = pds.tile([128, D], F32, tag="o")
                    nc.vector.tensor_scalar(
                        o[:], r2[:], scalar1=mv[:, 0:1], scalar2=rstd[:],
                        op0=mybir.AluOpType.subtract, op1=mybir.AluOpType.mult)
                    if "g2b2" in flags:
                        nc.vector.tensor_mul(o[:], o[:], g2_sb[:])
                        nc.vector.tensor_add(o[:], o[:], bt2_sb[:])
                    nc.gpsimd.dma_start(OUTr[qt], o[:])

                for hb in range(HB):
                    w2_hb = pwD.tile([128, 8, D], MMDT, tag="w",
                                     name=f"w2_{qtg}_{hb}")
                    nc.sync.dma_start(w2_hb[:], W2r[:, hb * 8:(hb + 1) * 8, :])
                    last = hb == HB - 1
                    for lqt in range(g1 - g0):
                        qt = g0 + lqt
                        for dhh in range(2):
                            for hti in range(8):
                                nc.tensor.matmul(
                                    p2[lqt][dhh][:],
                                    fsb[:, hb * 8 + hti,
                                        qt * 128:(qt + 1) * 128],
                                    w2_hb[:, hti, dhh * 512:(dhh + 1) * 512],
                                    start=(hb == 0 and hti == 0),
                                    stop=(last and hti == 7),
                                    skip_group_check=True)
                        if last:
                            # drain each qt as soon as its accumulation stops
                            # so the LN/output overlaps the remaining MMs
                            drain(lqt)

    nc.compile()
    return nc


def _get_program(flags, debug=None):
    key = (flags, debug)
    if key not in _BUILD_CACHE:
        _BUILD_CACHE[key] = _build(flags, debug)
    return _BUILD_CACHE[key]


def _mm_np(a):
    import ml_dtypes
    return np.ascontiguousarray(a, dtype=ml_dtypes.bfloat16)


def _f8_np(a, scale=1.0):
    import ml_dtypes
    if scale != 1.0:
        a = np.asarray(a, np.float32) * scale
    return np.ascontiguousarray(a, dtype=ml_dtypes.float8_e4m3fn)


def _make_in_maps(X, shared):
    in_maps = []
    for c in range(N_CORES):
        b, half = c // 2, c % 2
        xq = np.ascontiguousarray(X[b, half * SQ:(half + 1) * SQ])
        m = dict(shared)
        # XQ (residual) is pre-scaled by WS to match po = ctx @ (WS*Wo);
        # LN1 is scale-invariant so Y is unchanged
        m.update({"XT": _f8_np(X[b].T),
                  "XQT": _f8_np(xq.T), "XQ": _mm_np(xq * WS)})
        in_maps.append(m)
    return in_maps


def kernel(X, Wq, bq, Wk, bk, Wv, bv, Wo, bo, g1, beta1, W1, b1, W2, b2, g2,
           beta2, _debug=None, _trace=False):
    f32 = lambda a: np.ascontiguousarray(np.asarray(a), dtype=np.float32)
    X = f32(X)
    Wq, Wk, Wv, Wo, W1, W2 = map(f32, (Wq, Wk, Wv, Wo, W1, W2))
    bq, bk, bv, bo, b1, b2 = map(f32, (bq, bk, bv, bo, b1, b2))
    g1, beta1, g2, beta2 = map(f32, (g1, beta1, g2, beta2))

    flags = set()
    if bq.any() or bk.any() or bv.any():
        flags.add("bqkv")
    if bo.any():
        flags.add("bo")
    if b1.any():
        flags.add("b1")
    if b2.any():
        flags.add("b2")
    if (g1 != 1).any() or beta1.any():
        flags.add("g1b1")
    if (g2 != 1).any() or beta2.any():
        flags.add("g2b2")
    flags = frozenset(flags)

    nc = _get_program(flags, _debug)

    shared = {"WQ": _f8_np(Wq, WS), "WK": _f8_np(Wk, WS),
              "WV": _f8_np(Wv, WS), "WO": _f8_np(Wo, WS),
              "W1": _mm_np(W1), "W2": _mm_np(W2)}
    if "bqkv" in flags:
        shared.update({"BQ": bq, "BK": bk, "BV": bv})
    if "bo" in flags:
        shared["BO"] = bo
    if "b1" in flags:
        shared["B1"] = b1
    if "b2" in flags:
        shared["B2"] = b2
    if "g1b1" in flags:
        shared.update({"G1": g1, "BT1": beta1})
    if "g2b2" in flags:
        shared.update({"G2": g2, "BT2": beta2})

    in_maps = _make_in_maps(X, shared)
    res = run_bass_kernel_spmd(nc, in_maps, core_ids=list(range(N_CORES)),
                               trace=_trace)

    if _debug is not None or _trace:
        return res

    out = np.empty((B, S, D), dtype=np.float32)
    for c in range(N_CORES):
        b, half = c // 2, c % 2
        out[b, half * SQ:(half + 1) * SQ] = res.results[c]["OUT"]
    return out

